# revision 42
# baseline (speedup 1.0000x reference)
"""Trainium2 Bass kernel for nn_AttentionLayer_47596827574368.

Reference computation (per batch sample b, B=8, C=768, H=W=64, L=4096, Cqk=Cv=96):
  Q = Wq @ X, K = Wk @ X, V = Wv @ X            (X = x[b] as [C, L])
  S = Q^T K   [L, L];  beta = softmax(S, axis=-1)
  O = beta @ V^T      [L, Cv]
  y = gamma * (Wlast @ O^T) + X                 [C, L]

Sharding: data-parallel over batch — one sample per NeuronCore (8 cores).

Device plan (per core):
  X streamed in (chunk, 512-col) pieces; Q/K c-major [96, 4096]; V^T as 32
  blocks [128(k), 97] (col 96 = ones -> softmax denominators ride along in
  the attnV matmul); scores computed transposed S^T[k, q] per 128-k block.
  Softmax uses a global-shift exp (C = est_max + 8 sampled from k-block 0;
  exact per-row max is unnecessary: softmax is shift-invariant and fp32 exp
  has huge dynamic-range headroom). Normalization is applied before the
  final projection; gamma is folded into Wlast on the host.

  q chunk 0 (512 wide) is interleaved with the projection phase (its exps
  ride free under the input-DMA/projection span; phase 1 is co-bound by
  input DMA ~38us and PE ~46us, so PE-side work there is effectively
  free). Chunks 1-6 are 512 wide and chunks 7-8 are 256 wide (short
  un-overlappable tail); their k loops process kb-triples with a single
  [128, 3w] exp per triple — the widest exp the 8 PSUM banks allow
  (3-bank score tiles x2 bufs + shared out/z tag) — with a lag-2 triple
  FIFO so PE never waits on ScalarE. Each chunk's final-projection/
  residual work is spread through the next chunk's k loop. Matmuls run in
  float32r (full PE rate at moving>=256, ~1.5e-4 rel err).
"""

import numpy as np

import concourse.bass as bass
import concourse.tile as tile
import concourse.mybir as mybir
from concourse import bacc
from concourse import bass_utils
from concourse.masks import make_identity

F32 = mybir.dt.float32
F32R = mybir.dt.float32r
BF16 = mybir.dt.bfloat16
AF = mybir.ActivationFunctionType
AX = mybir.AxisListType

C = 768          # input/output channels
CQ = 96          # qk/v channels
L = 4096         # H*W
KC = C // 128    # 6 contraction chunks
NKB = L // 128   # 32 k blocks
MARGIN = 8.0     # exp shift safety margin

# chunk 0 (512) rides under the projection phase; 512-wide middle chunks;
# two 256-wide tail chunks keep the final drain short
CHUNKS = [(512 * i, 512) for i in range(7)] + [(3584, 256), (3840, 256)]


def kb_groups(ci):
    # kb-triple grouping for chunk ci's k loop (chunk 1's kbs 0,1 are
    # bridged across the PSUM pool swap)
    kbs = list(range(2, NKB)) if ci == 1 else list(range(NKB))
    return [kbs[i : i + 3] for i in range(0, len(kbs), 3)]


def body(nc, tc, sbuf, x, wqkv_t, wl_t, y):
    # ---- persistent sbuf tiles -----------------------------------------
    w_sb = sbuf.tile([128, KC, 3 * CQ], F32R, tag="w")
    x_sb = sbuf.tile([128, KC, L], F32R, tag="x")
    x_r = x.rearrange("(ko ki) l -> ki ko l", ki=128).bitcast(F32R)
    w_r = wqkv_t.rearrange("(ko ki) m -> ki ko m", ki=128).bitcast(F32R)
    # startup-critical DMA order: interleave weight and x(gp0) pieces per
    # kc so the first projection matmul starts early
    nc.sync.dma_start(out=w_sb[:, 0, :], in_=w_r[:, 0, :])
    nc.sync.dma_start(out=x_sb[:, 0, 0:512], in_=x_r[:, 0, 0:512])
    for kc in range(1, KC):
        nc.sync.dma_start(out=w_sb[:, kc, :], in_=w_r[:, kc, :])
        nc.sync.dma_start(out=x_sb[:, kc, 0:512], in_=x_r[:, kc, 0:512])
    for gp in range(1, 8):
        gs = slice(gp * 512, (gp + 1) * 512)
        for kc in range(KC):
            nc.sync.dma_start(out=x_sb[:, kc, gs], in_=x_r[:, kc, gs])
    wl_sb = sbuf.tile([CQ, C], F32R, tag="wl")
    nc.sync.dma_start(out=wl_sb, in_=wl_t.bitcast(F32R))

    ident = sbuf.tile([128, 128], F32, tag="ident")
    make_identity(nc, ident)
    ident_bf = sbuf.tile([128, 128], BF16, tag="identbf")
    make_identity(nc, ident_bf)

    q_sb = sbuf.tile([CQ, L], F32R, tag="q")
    k_sb = sbuf.tile([CQ, L], F32R, tag="k")
    v_sb = sbuf.tile([CQ, L], BF16, tag="vbig")
    vt_sb = sbuf.tile([128, NKB, CQ + 1], BF16, tag="vt")
    # ones column (f32r producer required: memset can't write f32r)
    nc.scalar.activation(
        out=vt_sb[:, :, CQ : CQ + 1].rearrange("p a b -> p (a b)"),
        in_=ident[:, 0:NKB],
        func=AF.Copy,
        bias=1.0,
        scale=0.0,
    )

    small = sbuf.tile([128, 16], F32, tag="small")
    m_row = small[:, 8:9]
    neg_c = small[:, 9:10]
    gmax_bc = small[:, 10:11]
    mt_sb = sbuf.tile([1, 128], F32, tag="rcp")

    attn_sb = sbuf.tile([CQ, L], F32R, tag="vbig", name="attn_sb")
    rcp_bc = sbuf.tile([CQ, 512], F32, tag="rbc")
    y_r = y.rearrange("(ko ki) l -> ki ko l", ki=128)

    def score_mm(s_ps, kb, c0, w):
        # s_ps slice is [128, w] with w <= 512 (single PSUM bank)
        nc.tensor.matmul(
            s_ps,
            k_sb[:, kb * 128 : (kb + 1) * 128],
            q_sb[:, c0 : c0 + w],
            start=True,
            stop=True,
        )

    def attnv_mm(out_ps, et, kb, w):
        nc.tensor.matmul(
            out_ps[0 : CQ + 1, 0:w],
            vt_sb[:, kb, :],
            et,
            start=(kb == 0),
            stop=(kb == NKB - 1),
        )

    def normalize(ci, out_ps):
        # stage out_ps to SBUF in one copy so its PSUM banks free fast, then
        #   attn[:, c0:c0+w] = stage[0:96] * (1 / stage[96])
        c0, w = CHUNKS[ci]
        ostage = sbuf.tile([CQ + 1, 512], F32, tag="ostage", bufs=2,
                           name=f"ostage_{ci}")[:, 0:w]
        nc.vector.tensor_copy(ostage, out_ps[0 : CQ + 1, 0:w])
        rcp_sb = sbuf.tile([1, 512], F32, tag="rcp", name=f"rcp_{ci}")[:, 0:w]
        nc.vector.reciprocal(rcp_sb, ostage[CQ : CQ + 1, :])
        nc.gpsimd.partition_broadcast(rcp_bc[:, 0:w], rcp_sb)
        nc.vector.tensor_mul(attn_sb[:, c0 : c0 + w], ostage[0:CQ, :], rcp_bc[:, 0:w])

    def phase4_unit(ps_pool, ci, oc):
        # final projection + residual for one 128-row output chunk; z tiles
        # share the accumulator tag's slots (transient between long-lived
        # accumulator lifetimes)
        c0, w = CHUNKS[ci]
        y_sb = sbuf.tile([128, 512], F32, tag="y", bufs=3,
                         name=f"y_sb_{ci}_{oc}")[:, 0:w]
        gs = slice(c0, c0 + w)
        z_ps = ps_pool.tile([128, w], F32, tag="oz", bufs=2,
                            name=f"z_ps_{ci}_{oc}", padded_shape=[128, 512])
        nc.tensor.matmul(
            z_ps,
            wl_sb[:, oc * 128 : (oc + 1) * 128],
            attn_sb[:, gs],
            start=True,
            stop=True,
        )
        nc.vector.tensor_add(y_sb, z_ps, x_sb[:, oc, gs].bitcast(F32))
        nc.sync.dma_start(out=y_r[:, oc, gs], in_=y_sb)

    # ---- phase 1 + attention chunk 0 (512 wide), interleaved ------------
    # projections run in 512-column groups; as each group's K/V land, the
    # corresponding k-blocks of chunk 0 are scored/exp'd/accumulated.
    with (
        tc.tile_pool(name="ps_proj", bufs=1, space="PSUM") as ps_proj,
        tc.tile_pool(name="ps_aux", bufs=2, space="PSUM") as ps_aux,
    ):
        out0_ps = ps_proj.tile([128, 512], F32, tag="o0", name="out0_ps")
        pend_attnv = []  # two-kb lag FIFO so PE never waits on ACT in-order
        for gp in range(8):
            gs = slice(gp * 512, (gp + 1) * 512)
            tiles = [
                ps_proj.tile([CQ, 512], F32, tag=f"proj{t}", name=f"p_ps_{t}_{gp}")
                for t in range(3)
            ]
            for kc in range(KC):
                for t in range(3):
                    nc.tensor.matmul(
                        tiles[t],
                        w_sb[:, kc, t * CQ : (t + 1) * CQ],
                        x_sb[:, kc, gs],
                        start=(kc == 0),
                        stop=(kc == KC - 1),
                    )
            for t, dst in ((0, q_sb), (1, k_sb), (2, v_sb)):
                if t == 1:
                    nc.vector.tensor_copy(dst[:, gs], tiles[t])
                else:
                    nc.scalar.copy(dst[:, gs], tiles[t])

            # V -> V^T transposes for this group's 4 l-blocks
            for lb in range(4 * gp, 4 * gp + 4):
                t_ps = ps_aux.tile([128, CQ], BF16, tag="sm", name=f"t_ps_{lb}")
                nc.tensor.transpose(
                    t_ps, v_sb[:, lb * 128 : (lb + 1) * 128], ident_bf[0:CQ, 0:CQ]
                )
                nc.vector.tensor_copy(vt_sb[:, lb, 0:CQ], t_ps)

            # chunk-0 attention for this group's 4 k-blocks
            for kb in range(4 * gp, 4 * gp + 4):
                s_ps = ps_proj.tile([128, 512], F32, tag="s0", bufs=2,
                                    name=f"s_ps_0_{kb}")
                score_mm(s_ps, kb, 0, 512)
                if kb == 0:
                    # shift estimate from these 65k scores (statistically
                    # ample for a shift that merely has to land within
                    # ~+-80 of the true max)
                    nc.vector.reduce_max(m_row, s_ps, axis=AX.X)
                    mt_ps = ps_aux.tile([1, 128], F32, tag="sm")
                    nc.tensor.transpose(mt_ps, m_row, ident)
                    nc.vector.tensor_copy(mt_sb[:, 0:128], mt_ps)
                    nc.vector.reduce_max(small[0:1, 11:12], mt_sb[:, 0:128],
                                         axis=AX.X)
                    nc.gpsimd.partition_broadcast(gmax_bc, small[0:1, 11:12])
                    # neg_c = -(gmax + MARGIN)
                    nc.scalar.activation(neg_c, gmax_bc, AF.Copy,
                                         bias=-MARGIN, scale=-1.0)
                et = sbuf.tile([128, 1536], BF16, tag="et", bufs=4,
                               name=f"et_0_{kb}")[:, 0:512]
                nc.scalar.activation(et, s_ps, AF.Exp, bias=neg_c, scale=1.0)
                if len(pend_attnv) >= 2:
                    pa = pend_attnv.pop(0)
                    attnv_mm(out0_ps, pa[0], pa[1], 512)
                pend_attnv.append((et, kb))
        for pa in pend_attnv:
            attnv_mm(out0_ps, pa[0], pa[1], 512)
        # bridge: score+exp chunk-1's k-blocks 0,1 in this pool's slots so
        # ScalarE never idles across the PSUM pool swap
        bridge = []
        for kb in (0, 1):
            sb_ps = ps_proj.tile([128, 512], F32, tag="s0", bufs=2,
                                 name=f"sb_ps_{kb}")
            score_mm(sb_ps, kb, 512, 512)
            bet = sbuf.tile([128, 1536], BF16, tag="et", bufs=4,
                            name=f"et_1_{kb}")[:, 0:512]
            nc.scalar.activation(bet, sb_ps, AF.Exp, bias=neg_c, scale=1.0)
            bridge.append([(bet, kb, 512)])
        normalize(0, out0_ps)

    # ---- attention chunks 1..8 ------------------------------------------
    # per chunk: kb-triples; one [128, 3w] score tile + one exp per triple;
    # lag-2 triple FIFO for attnV; phase4(ci-1) spread mid-loop.
    with tc.tile_pool(name="ps_attn", bufs=1, space="PSUM") as ps_attn:
        for ci in range(1, len(CHUNKS)):
            c0, w = CHUNKS[ci]
            out_ps = ps_attn.tile(
                [CQ + 1, 512], F32, tag="oz", bufs=2, name=f"out_ps_{ci}"
            )
            pend = list(bridge) if ci == 1 else []
            groups = kb_groups(ci)
            ph4_at = {2, 3, 4, 5, 6, 7}
            for gi, grp in enumerate(groups):
                s_ps = ps_attn.tile(
                    [128, 3 * w], F32, tag="s3", bufs=2,
                    name=f"s_ps_{ci}_{gi}", padded_shape=[128, 1536]
                )
                for i, kb in enumerate(grp):
                    score_mm(s_ps[:, w * i : w * (i + 1)], kb, c0, w)
                gw = w * len(grp)
                et = sbuf.tile([128, 1536], BF16, tag="et", bufs=4,
                               name=f"et_{ci}_{gi}")[:, 0:gw]
                nc.scalar.activation(et, s_ps[:, 0:gw], AF.Exp,
                                     bias=neg_c, scale=1.0)
                if len(pend) >= 2:
                    for pe, pk, pw in pend.pop(0):
                        attnv_mm(out_ps, pe, pk, pw)
                pend.append(
                    [(et[:, w * i : w * (i + 1)], kb, w)
                     for i, kb in enumerate(grp)]
                )
                # spread the previous chunk's phase 4 through this chunk's
                # k loop (keeps the DVE queue shallow so the boundary chain
                # is never stuck behind it)
                if gi in ph4_at:
                    phase4_unit(ps_attn, ci - 1, gi - 2)
            for ets in pend:
                for pe, pk, pw in ets:
                    attnv_mm(out_ps, pe, pk, pw)
            normalize(ci, out_ps)

        # last chunk's phase 4: nothing overlaps it, so avoid the per-unit
        # z->add->dma semaphore chains — batch 3 output chunks per s3-tagged
        # PSUM tile (free once scores stop), one fused add, one DMA
        c0, w = CHUNKS[-1]
        gs = slice(c0, c0 + w)
        for h in range(2):
            ocs = range(3 * h, 3 * h + 3)
            zball = ps_attn.tile([128, 3, w], F32, tag="s3", bufs=2,
                                 name=f"zball_{h}", padded_shape=[128, 3, 512])
            for i, oc in enumerate(ocs):
                nc.tensor.matmul(
                    zball[:, i, :],
                    wl_sb[:, oc * 128 : (oc + 1) * 128],
                    attn_sb[:, gs],
                    start=True,
                    stop=True,
                )
            yball = sbuf.tile([128, 3, w], F32, tag="yball", bufs=2,
                              name=f"yball_{h}")
            nc.vector.tensor_add(yball, zball,
                                 x_sb[:, 3 * h : 3 * h + 3, gs].bitcast(F32))
            nc.sync.dma_start(out=y_r[:, 3 * h : 3 * h + 3, gs], in_=yball)


def build(loop_iters=1):
    nc = bacc.Bacc("TRN2", target_bir_lowering=False, debug=False, num_devices=8)
    x = nc.dram_tensor("x", [C, L], F32, kind="ExternalInput").ap()
    wqkv_t = nc.dram_tensor("wqkv_t", [C, 3 * CQ], F32, kind="ExternalInput").ap()
    wl_t = nc.dram_tensor("wl_t", [CQ, C], F32, kind="ExternalInput").ap()
    y = nc.dram_tensor("y", [C, L], F32, kind="ExternalOutput").ap()

    with tile.TileContext(nc) as tc:
        with tc.tile_pool(name="sbuf", bufs=1) as sbuf:
            if loop_iters > 1:
                engines = (
                    mybir.EngineType.PE,
                    mybir.EngineType.Activation,
                    mybir.EngineType.DVE,
                    mybir.EngineType.Pool,
                    mybir.EngineType.SP,
                )
                with tc.For_i(0, loop_iters, hint_engines=engines):
                    body(nc, tc, sbuf, x, wqkv_t, wl_t, y)
            else:
                body(nc, tc, sbuf, x, wqkv_t, wl_t, y)

    nc.compile()
    return nc


_cached_nc = None


def kernel(x, Wq, Wk, Wv, Wlast, gamma):
    global _cached_nc
    x = np.ascontiguousarray(np.asarray(x, dtype=np.float32))
    B = x.shape[0]
    assert B == 8 and x.shape[1:] == (C, 64, 64)
    wqkv_t = np.ascontiguousarray(
        np.concatenate([Wq, Wk, Wv], axis=0).T.astype(np.float32)
    )
    wl_t = np.ascontiguousarray(
        (np.asarray(Wlast, np.float32) * np.float32(np.asarray(gamma)[0])).T
    )

    if _cached_nc is None:
        _cached_nc = build()
    nc = _cached_nc

    in_maps = [
        {
            "x": np.ascontiguousarray(x[b].reshape(C, L)),
            "wqkv_t": wqkv_t,
            "wl_t": wl_t,
        }
        for b in range(B)
    ]
    res = bass_utils.run_bass_kernel_spmd(nc, in_maps, core_ids=list(range(B)))
    out = np.stack([res.results[b]["y"].reshape(C, 64, 64) for b in range(B)])
    return out.astype(np.float32)


# revision 43
# speedup vs baseline: 1.3247x; 1.3247x over previous
"""Trainium2 Bass kernel for nn_AttentionLayer_47596827574368.

Reference computation (per batch sample b, B=8, C=768, H=W=64, L=4096, Cqk=Cv=96):
  Q = Wq @ X, K = Wk @ X, V = Wv @ X            (X = x[b] as [C, L])
  S = Q^T K   [L, L];  beta = softmax(S, axis=-1)
  O = beta @ V^T      [L, Cv]
  y = gamma * (Wlast @ O^T) + X                 [C, L]

Sharding: data-parallel over batch — one sample per NeuronCore (8 cores).

Device plan (per core):
  X streamed in (chunk, 512-col) pieces; Q/K c-major [96, 4096]; V^T as 32
  blocks [128(k), 97] (col 96 = ones -> softmax denominators ride along in
  the attnV matmul); scores computed transposed S^T[k, q] per 128-k block.
  Softmax uses a global-shift exp (C = est_max + 8 sampled from k-block 0;
  exact per-row max is unnecessary: softmax is shift-invariant and fp32 exp
  has huge dynamic-range headroom). Normalization is applied before the
  final projection; gamma is folded into Wlast on the host.

  q chunk 0 (512 wide) is interleaved with the projection phase (its exps
  ride free under the input-DMA/projection span; phase 1 is co-bound by
  input DMA ~38us and PE ~46us, so PE-side work there is effectively
  free). Chunks 1-6 are 512 wide and chunks 7-8 are 256 wide (short
  un-overlappable tail); their k loops process kb-triples with a single
  [128, 3w] exp per triple — the widest exp the 8 PSUM banks allow
  (3-bank score tiles x2 bufs + shared out/z tag) — with a lag-2 triple
  FIFO so PE never waits on ScalarE. Each chunk's final-projection/
  residual work is spread through the next chunk's k loop. Matmuls run in
  float32r (full PE rate at moving>=256, ~1.5e-4 rel err).
"""

import numpy as np

import concourse.bass as bass
import concourse.tile as tile
import concourse.mybir as mybir
from concourse import bacc
from concourse import bass_utils
from concourse import bass_isa
from concourse.masks import make_identity

F32 = mybir.dt.float32
F32R = mybir.dt.float32r
BF16 = mybir.dt.bfloat16
AF = mybir.ActivationFunctionType
AX = mybir.AxisListType

C = 768          # input/output channels
CQ = 96          # qk/v channels
L = 4096         # H*W
KC = C // 128    # 6 contraction chunks
NKB = L // 128   # 32 k blocks
MARGIN = 8.0     # exp shift safety margin

# chunk 0 (512) rides under the projection phase; 512-wide middle chunks;
# two 256-wide tail chunks keep the final drain short
CHUNKS = [(512 * i, 512) for i in range(7)] + [(3584, 256), (3840, 256)]


def unit_groups():
    # global (ci, kb) unit stream over chunks 1..8 (chunk 1's kbs 0,1 are
    # bridged across the PSUM pool swap), packed greedily into exp groups
    # of total width <= 1536 regardless of chunk boundaries: 63 triples of
    # 512, one mixed [512, 256x4] group, ten 6x256 groups — every exp is
    # full width
    units = []
    for ci in range(1, len(CHUNKS)):
        for kb in range(2, NKB) if ci == 1 else range(NKB):
            units.append((ci, kb))
    groups = []
    cur, cur_w = [], 0
    for ci, kb in units:
        w = CHUNKS[ci][1]
        if cur_w + w > 1536:
            groups.append(cur)
            cur, cur_w = [], 0
        cur.append((ci, kb))
        cur_w += w
    groups.append(cur)
    return groups


def body(nc, tc, sbuf, x, wqkv_t, wl_t, y):
    # ---- persistent sbuf tiles -----------------------------------------
    w_sb = sbuf.tile([128, KC, 3 * CQ], F32R, tag="w")
    x_sb = sbuf.tile([128, KC, L], F32R, tag="x")
    x_r = x.rearrange("(ko ki) l -> ki ko l", ki=128).bitcast(F32R)
    w_r = wqkv_t.rearrange("(ko ki) m -> ki ko m", ki=128).bitcast(F32R)
    # startup-critical DMA order: interleave weight and x(gp0) pieces per
    # kc so the first projection matmul starts early
    nc.sync.dma_start(out=w_sb[:, 0, :], in_=w_r[:, 0, :])
    nc.sync.dma_start(out=x_sb[:, 0, 0:512], in_=x_r[:, 0, 0:512])
    for kc in range(1, KC):
        nc.sync.dma_start(out=w_sb[:, kc, :], in_=w_r[:, kc, :])
        nc.sync.dma_start(out=x_sb[:, kc, 0:512], in_=x_r[:, kc, 0:512])
    for gp in range(1, 8):
        gs = slice(gp * 512, (gp + 1) * 512)
        for kc in range(KC):
            nc.sync.dma_start(out=x_sb[:, kc, gs], in_=x_r[:, kc, gs])
    wl_sb = sbuf.tile([CQ, C], F32R, tag="wl")
    nc.sync.dma_start(out=wl_sb, in_=wl_t.bitcast(F32R))

    ident_bf = sbuf.tile([128, 128], BF16, tag="identbf")
    make_identity(nc, ident_bf)

    q_sb = sbuf.tile([CQ, L], F32R, tag="q")
    k_sb = sbuf.tile([CQ, L], F32R, tag="k")
    v_sb = sbuf.tile([CQ, L], BF16, tag="vbig")
    vt_sb = sbuf.tile([128, NKB, CQ + 1], BF16, tag="vt")
    # ones column for the denominator ride-along
    nc.vector.memset(
        vt_sb[:, :, CQ : CQ + 1].rearrange("p a b -> p (a b)"), 1.0
    )

    small = sbuf.tile([128, 16], F32, tag="small")
    m_row = small[:, 8:9]
    m_all = small[:, 10:11]
    neg_c = small[:, 9:10]

    attn_sb = sbuf.tile([CQ, L], F32R, tag="vbig", name="attn_sb")
    rcp_bc = sbuf.tile([CQ, 512], F32, tag="rbc")
    y_r = y.rearrange("(ko ki) l -> ki ko l", ki=128)

    def score_mm(s_ps, kb, c0, w):
        # s_ps slice is [128, w] with w <= 512 (single PSUM bank)
        nc.tensor.matmul(
            s_ps,
            k_sb[:, kb * 128 : (kb + 1) * 128],
            q_sb[:, c0 : c0 + w],
            start=True,
            stop=True,
        )

    def attnv_mm(out_ps, et, kb, w):
        nc.tensor.matmul(
            out_ps[0 : CQ + 1, 0:w],
            vt_sb[:, kb, :],
            et,
            start=(kb == 0),
            stop=(kb == NKB - 1),
        )

    def normalize(ci, out_ps):
        # stage out_ps to SBUF in one copy so its PSUM banks free fast, then
        #   attn[:, c0:c0+w] = stage[0:96] * (1 / stage[96])
        c0, w = CHUNKS[ci]
        ostage = sbuf.tile([CQ + 1, 512], F32, tag="ostage", bufs=2,
                           name=f"ostage_{ci}")[:, 0:w]
        nc.vector.tensor_copy(ostage, out_ps[0 : CQ + 1, 0:w])
        rcp_sb = sbuf.tile([1, 512], F32, tag="rcp", name=f"rcp_{ci}")[:, 0:w]
        nc.vector.reciprocal(rcp_sb, ostage[CQ : CQ + 1, :])
        nc.gpsimd.partition_broadcast(rcp_bc[:, 0:w], rcp_sb)
        nc.vector.tensor_mul(attn_sb[:, c0 : c0 + w], ostage[0:CQ, :], rcp_bc[:, 0:w])

    def phase4_unit(ps_pool, ci, oc):
        # final projection + residual for one 128-row output chunk; z tiles
        # share the accumulator tag's slots (transient between long-lived
        # accumulator lifetimes)
        c0, w = CHUNKS[ci]
        y_sb = sbuf.tile([128, 512], F32, tag="y", bufs=3,
                         name=f"y_sb_{ci}_{oc}")[:, 0:w]
        gs = slice(c0, c0 + w)
        z_ps = ps_pool.tile([128, w], F32, tag="oz", bufs=2,
                            name=f"z_ps_{ci}_{oc}", padded_shape=[128, 512])
        nc.tensor.matmul(
            z_ps,
            wl_sb[:, oc * 128 : (oc + 1) * 128],
            attn_sb[:, gs],
            start=True,
            stop=True,
        )
        nc.vector.tensor_add(y_sb, z_ps, x_sb[:, oc, gs].bitcast(F32))
        nc.sync.dma_start(out=y_r[:, oc, gs], in_=y_sb)

    # ---- phase 1 + attention chunk 0 (512 wide), interleaved ------------
    # projections run in 512-column groups; as each group's K/V land, the
    # corresponding k-blocks of chunk 0 are scored/exp'd/accumulated.
    with (
        tc.tile_pool(name="ps_proj", bufs=1, space="PSUM") as ps_proj,
        tc.tile_pool(name="ps_aux", bufs=2, space="PSUM") as ps_aux,
    ):
        out0_ps = ps_proj.tile([128, 512], F32, tag="o0", name="out0_ps")
        pend_attnv = []  # two-kb lag FIFO so PE never waits on ACT in-order
        for gp in range(8):
            gs = slice(gp * 512, (gp + 1) * 512)
            tiles = [
                ps_proj.tile([CQ, 512], F32, tag=f"proj{t}", name=f"p_ps_{t}_{gp}")
                for t in range(3)
            ]
            for kc in range(KC):
                for t in range(3):
                    nc.tensor.matmul(
                        tiles[t],
                        w_sb[:, kc, t * CQ : (t + 1) * CQ],
                        x_sb[:, kc, gs],
                        start=(kc == 0),
                        stop=(kc == KC - 1),
                    )
            for t, dst in ((0, q_sb), (1, k_sb), (2, v_sb)):
                if t == 1:
                    nc.vector.tensor_copy(dst[:, gs], tiles[t])
                else:
                    nc.scalar.copy(dst[:, gs], tiles[t])

            # V -> V^T transposes for this group's 4 l-blocks
            for lb in range(4 * gp, 4 * gp + 4):
                t_ps = ps_aux.tile([128, CQ], BF16, tag="sm", name=f"t_ps_{lb}")
                nc.tensor.transpose(
                    t_ps, v_sb[:, lb * 128 : (lb + 1) * 128], ident_bf[0:CQ, 0:CQ]
                )
                nc.vector.tensor_copy(vt_sb[:, lb, 0:CQ], t_ps)

            # chunk-0 attention for this group's 4 k-blocks
            for kb in range(4 * gp, 4 * gp + 4):
                s_ps = ps_proj.tile([128, 512], F32, tag="s0", bufs=2,
                                    name=f"s_ps_0_{kb}")
                score_mm(s_ps, kb, 0, 512)
                if kb == 0:
                    # shift estimate from a 16k-score sample: exp's fp32/
                    # bf16 dynamic range dwarfs the max-estimate shortfall,
                    # so a small early sample minimizes the neg_c latency
                    # that gates the first exps
                    nc.vector.reduce_max(m_row, s_ps[:, 0:128], axis=AX.X)
                    nc.gpsimd.partition_all_reduce(
                        m_all, m_row, channels=128,
                        reduce_op=bass_isa.ReduceOp.max,
                    )
                    # neg_c = -(gmax + MARGIN)
                    nc.scalar.activation(neg_c, m_all, AF.Copy,
                                         bias=-MARGIN, scale=-1.0)
                et = sbuf.tile([128, 1536], BF16, tag="et", bufs=4,
                               name=f"et_0_{kb}")[:, 0:512]
                nc.scalar.activation(et, s_ps, AF.Exp, bias=neg_c, scale=1.0)
                if len(pend_attnv) >= 2:
                    pa = pend_attnv.pop(0)
                    attnv_mm(out0_ps, pa[0], pa[1], 512)
                pend_attnv.append((et, kb))
        for pa in pend_attnv:
            attnv_mm(out0_ps, pa[0], pa[1], 512)
        # bridge: score+exp chunk-1's k-blocks 0,1 in this pool's slots so
        # ScalarE never idles across the PSUM pool swap
        bridge = []
        for kb in (0, 1):
            sb_ps = ps_proj.tile([128, 512], F32, tag="s0", bufs=2,
                                 name=f"sb_ps_{kb}")
            score_mm(sb_ps, kb, 512, 512)
            bet = sbuf.tile([128, 1536], BF16, tag="et", bufs=4,
                            name=f"et_1_{kb}")[:, 0:512]
            nc.scalar.activation(bet, sb_ps, AF.Exp, bias=neg_c, scale=1.0)
            bridge.append([(bet, kb, 512)])
        normalize(0, out0_ps)

    # ---- attention chunks 1..8, software-pipelined across boundaries ----
    # One flat group stream: kb-triples; one [128, 3w] score tile + one exp
    # per triple; a lag-2 group FIFO for attnV that CROSSES chunk
    # boundaries (each pend entry carries its own accumulator), so the next
    # chunk's scores are never gated behind the previous chunk's attnV
    # drain + normalize chain. normalize(c) is emitted when c's last group
    # pops, ~2 groups into chunk c+1; phase4(ci-1) spreads at groups 3-8.
    with tc.tile_pool(name="ps_attn", bufs=1, space="PSUM") as ps_attn:
        pend = []  # (out_ps of unit, et slice, ci, kb, w)
        outs = {}  # ci -> accumulator tile

        def pop_one():
            for out_p, pe, pci, pk, pw in pend.pop(0):
                attnv_mm(out_p, pe, pk, pw)
                if pk == NKB - 1:
                    normalize(pci, out_p)

        # phase4(ci-1) spread points: at emission of (ci, kb) for these
        # kbs, normalize(ci-1) (which pops ~6 kbs back) is surely done
        PH4_AT = {16: 0, 18: 1, 20: 2, 22: 3, 24: 4, 26: 5}

        for gi, grp in enumerate(unit_groups()):
            for ci, kb in grp:
                if ci not in outs:
                    outs[ci] = ps_attn.tile(
                        [CQ + 1, 512], F32, tag="oz", bufs=2,
                        name=f"out_ps_{ci}"
                    )
                    if ci == 1:
                        for ets in bridge:
                            pend.append([(outs[1], pe, 1, pk, pw)
                                         for pe, pk, pw in ets])
            s_ps = ps_attn.tile(
                [128, 1536], F32, tag="s3", bufs=2, name=f"s_ps_{gi}"
            )
            off = 0
            offs = []
            for ci, kb in grp:
                w = CHUNKS[ci][1]
                score_mm(s_ps[:, off : off + w], kb, CHUNKS[ci][0], w)
                offs.append(off)
                off += w
            et = sbuf.tile([128, 1536], BF16, tag="et", bufs=4,
                           name=f"et_{gi}")[:, 0:off]
            nc.scalar.activation(et, s_ps[:, 0:off], AF.Exp,
                                 bias=neg_c, scale=1.0)
            if len(pend) >= 2:
                pop_one()
            pend.append([
                (outs[ci], et[:, o : o + CHUNKS[ci][1]], ci, kb,
                 CHUNKS[ci][1])
                for (ci, kb), o in zip(grp, offs)
            ])
            for ci, kb in grp:
                if ci >= 2 and kb in PH4_AT:
                    phase4_unit(ps_attn, ci - 1, PH4_AT[kb])
                elif ci == 1 and kb in PH4_AT:
                    phase4_unit(ps_attn, 0, PH4_AT[kb])
        while pend:
            pop_one()

        # last chunk's phase 4: nothing overlaps it, so avoid the per-unit
        # z->add->dma semaphore chains — batch 3 output chunks per s3-tagged
        # PSUM tile (free once scores stop), one fused add, one DMA
        c0, w = CHUNKS[-1]
        gs = slice(c0, c0 + w)
        for h in range(3):
            ocs = range(2 * h, 2 * h + 2)
            zball = ps_attn.tile([128, 2, w], F32, tag="s3", bufs=2,
                                 name=f"zball_{h}", padded_shape=[128, 2, 512])
            for i, oc in enumerate(ocs):
                nc.tensor.matmul(
                    zball[:, i, :],
                    wl_sb[:, oc * 128 : (oc + 1) * 128],
                    attn_sb[:, gs],
                    start=True,
                    stop=True,
                )
            yball = sbuf.tile([128, 2, w], F32, tag="yball", bufs=3,
                              name=f"yball_{h}")
            nc.vector.tensor_add(yball, zball,
                                 x_sb[:, 2 * h : 2 * h + 2, gs].bitcast(F32))
            nc.sync.dma_start(out=y_r[:, 2 * h : 2 * h + 2, gs], in_=yball)


def build(loop_iters=1):
    nc = bacc.Bacc("TRN2", target_bir_lowering=False, debug=False, num_devices=8)
    x = nc.dram_tensor("x", [C, L], F32, kind="ExternalInput").ap()
    wqkv_t = nc.dram_tensor("wqkv_t", [C, 3 * CQ], F32, kind="ExternalInput").ap()
    wl_t = nc.dram_tensor("wl_t", [CQ, C], F32, kind="ExternalInput").ap()
    y = nc.dram_tensor("y", [C, L], F32, kind="ExternalOutput").ap()

    with tile.TileContext(nc) as tc:
        with tc.tile_pool(name="sbuf", bufs=1) as sbuf:
            if loop_iters > 1:
                engines = (
                    mybir.EngineType.PE,
                    mybir.EngineType.Activation,
                    mybir.EngineType.DVE,
                    mybir.EngineType.Pool,
                    mybir.EngineType.SP,
                )
                with tc.For_i(0, loop_iters, hint_engines=engines):
                    body(nc, tc, sbuf, x, wqkv_t, wl_t, y)
            else:
                body(nc, tc, sbuf, x, wqkv_t, wl_t, y)

    nc.compile()
    return nc


_cached_nc = None


def kernel(x, Wq, Wk, Wv, Wlast, gamma):
    global _cached_nc
    x = np.ascontiguousarray(np.asarray(x, dtype=np.float32))
    B = x.shape[0]
    assert B == 8 and x.shape[1:] == (C, 64, 64)
    wqkv_t = np.ascontiguousarray(
        np.concatenate([Wq, Wk, Wv], axis=0).T.astype(np.float32)
    )
    wl_t = np.ascontiguousarray(
        (np.asarray(Wlast, np.float32) * np.float32(np.asarray(gamma)[0])).T
    )

    if _cached_nc is None:
        _cached_nc = build()
    nc = _cached_nc

    in_maps = [
        {
            "x": np.ascontiguousarray(x[b].reshape(C, L)),
            "wqkv_t": wqkv_t,
            "wl_t": wl_t,
        }
        for b in range(B)
    ]
    res = bass_utils.run_bass_kernel_spmd(nc, in_maps, core_ids=list(range(B)))
    out = np.stack([res.results[b]["y"].reshape(C, 64, 64) for b in range(B)])
    return out.astype(np.float32)
